# revision 1
# baseline (speedup 1.0000x reference)
"""GCN layer (gnn_message_passing) on 8 Trainium2 NeuronCores.

out = relu(D^-1/2 (A+I) D^-1/2 (X W) + b)   with N=50000, E=500000,
F_IN=128, F_OUT=512.

Factorization: matmul by W commutes with the (linear) normalized
aggregation, so we aggregate in F_IN=128 feature space (4x less gather
traffic than aggregating X@W), then apply W once per output shard:

    h = D^-1/2 (A+I) D^-1/2 X ;  out = relu(h @ W + b)

Sharding: nodes are split 8 x 6250 (one shard per core); edges are
partitioned by destination shard so the scatter-add is core-local; x is
replicated (it is an input, so replication is free) and serves directly
as the gather table in HBM.

Per-core device pipeline:
  1. dma_gather of x rows (512B each) for this core's edges, in chunks.
     Source-node ids can exceed int16 (50000 > 32767), so each window's
     edge list is split into src<32768 (A) and src>=32768 (B, gathered
     from a base-shifted view of x).
  2. Per 128-edge tile: one fused DVE tensor_scalar builds the weighted
     selection matrix S[p,j] = (iota[j]==dest_off[p]) * dinv[src[p]]
     (fp32, 256-wide window); PE accumulates zT[win] += G.T @ S into a
     PSUM window. Both matmul inputs are bitcast to float32r: with the
     moving free dim >= 256 the PE streams 1 row/cycle (4x over fp32).
  3. Window flush: DVE multiplies by dinv[dst] -> hT tile (fp32).
  4. Phase C: PE hT.T @ W (float32r again, N=512) -> ACT relu -> DMA
     out. Overlaps with later windows' gathers.

Self-loops are added as real edges; dinv is computed host-side from the
edge index (pure index preprocessing: bincount + rsqrt); per-edge pad
slots carry dinv=0 so they contribute nothing.
"""
import os
import sys

for _p in ("/opt/trn_rl_repo",):
    if _p not in sys.path and os.path.isdir(_p):
        sys.path.insert(0, _p)

import numpy as np
import ml_dtypes

import concourse.bacc as bacc
import concourse.tile as tile
from concourse import mybir
from concourse.bass_utils import run_bass_kernel_spmd
from concourse.library_config import mlp as mlp_library

N = 50000
E = 500000
F_IN = 128
F_OUT = 512
NCORES = 8
SHARD = N // NCORES            # 6250
WIN = 256                      # PSUM window: dest nodes per window
NWIN = (SHARD + WIN - 1) // WIN  # 49
SPLIT = 32768                  # int16 gather-index limit
TILE = 128                     # edges per tile (partition dim)
GW = 2                         # windows per gather group
F32 = mybir.dt.float32
F32R = mybir.dt.float32r
F16 = mybir.dt.float16
I16 = mybir.dt.int16


def _prep(edge_index):
    """Host-side index preprocessing -> per-core input arrays + layout."""
    src = edge_index[0].astype(np.int64)
    dst = edge_index[1].astype(np.int64)

    deg = np.bincount(dst, minlength=N).astype(np.float64) + 1.0
    dinv = (1.0 / np.sqrt(deg)).astype(np.float32)

    loops = np.arange(N, dtype=np.int64)
    src = np.concatenate([src, loops])
    dst = np.concatenate([dst, loops])

    core = dst // SHARD
    local = dst - core * SHARD
    win = local // WIN
    off = local - win * WIN
    isB = (src >= SPLIT).astype(np.int64)

    key = (core * NWIN + win) * 2 + isB
    counts = np.bincount(key, minlength=NCORES * NWIN * 2).reshape(
        NCORES, NWIN, 2)
    ntiles = np.maximum((-(-counts // TILE)).max(axis=0), 1)  # [NWIN, 2]

    # tile stream: per window-group g: A tiles of wins in g, then B tiles.
    # One gather chunk per (window, list) segment piece (8-tile SWDGE ring
    # cap); only the first `reg` indices of each chunk generate descriptors
    # (trailing idx=-1 pads are skipped by the ucode). All cores share the
    # same reg (=max real count over cores, >=16), 0-filled per core.
    groups = [list(range(s, min(s + GW, NWIN))) for s in range(0, NWIN, GW)]
    tile_win, tile_list = [], []
    tstart = np.zeros((NWIN, 2), dtype=np.int64)
    zfill = np.zeros((NWIN, 2), dtype=np.int64)   # 0-fill extent per segment
    pos = 0
    chunks = []  # (list_id, start_tile, ntile, reg) per gather
    real_counts = counts.max(axis=0)  # [NWIN, 2]
    for g in groups:
        for l in (0, 1):
            for w in g:
                tstart[w, l] = pos
                seg = int(ntiles[w, l])
                reg_seg = max(16, int(real_counts[w, l]))
                for _ in range(seg):
                    tile_win.append(w)
                    tile_list.append(l)
                left = seg
                while left > 0:
                    take = min(left, 8)
                    done = (seg - left) * TILE
                    reg = min(max(reg_seg - done, 16), take * TILE)
                    chunks.append((l, pos + (seg - left), take, reg))
                    zfill[w, l] = max(zfill[w, l], done + reg)
                    left -= take
                pos += seg
    T = pos
    tile_win = np.array(tile_win)
    tile_list = np.array(tile_list)

    order = np.lexsort((isB, win, core))
    src_s, win_s, off_s, isB_s, core_s = (
        src[order], win[order], off[order], isB[order], core[order])
    dinv_src_all = dinv[src_s]

    per_core = []
    for c in range(NCORES):
        sel = core_s == c
        csrc, cdnv = src_s[sel], dinv_src_all[sel]
        coff = off_s[sel]
        idx_flat = np.full(T * TILE, -1, dtype=np.int64)
        off_flat = np.zeros(T * TILE, dtype=np.float32)
        dnv_flat = np.zeros(T * TILE, dtype=np.float32)
        start = 0
        for w in range(NWIN):
            for l in (0, 1):
                cnt = counts[c, w, l]
                s0 = tstart[w, l] * TILE
                idx_flat[s0:s0 + cnt] = csrc[start:start + cnt] - (
                    SPLIT if l else 0)
                off_flat[s0:s0 + cnt] = coff[start:start + cnt]
                dnv_flat[s0:s0 + cnt] = cdnv[start:start + cnt]
                idx_flat[s0 + cnt:s0 + int(zfill[w, l])] = 0
                start += cnt

        # wrapped int16 layout for dma_gather: idx j -> partition j%16,
        # col j//16, replicated across the 8 16-partition groups
        idx_w = np.tile(idx_flat.reshape(-1, 16).T.astype(np.int16), (8, 1))
        dest_off = off_flat.reshape(T, TILE).T.astype(np.float32)
        dinv_src = dnv_flat.reshape(T, TILE).T.astype(np.float32)
        dinvrep = np.tile(
            np.pad(dinv[c * SHARD:(c + 1) * SHARD],
                   (0, NWIN * WIN - SHARD)), (128, 1)).astype(np.float32)
        per_core.append(dict(idx=np.ascontiguousarray(idx_w),
                             dest=np.ascontiguousarray(dest_off),
                             dsrc=np.ascontiguousarray(dinv_src),
                             drep=dinvrep))

    layout = dict(T=T, tile_win=tile_win, tile_list=tile_list,
                  chunks=chunks, groups=groups, ntiles=ntiles,
                  tstart=tstart)
    return per_core, layout


def _build(layout, has_bias):
    T = layout["T"]
    tile_win = layout["tile_win"]
    chunks = layout["chunks"]
    groups = layout["groups"]
    ntiles = layout["ntiles"]
    tstart = layout["tstart"]

    nc = bacc.Bacc("TRN2", target_bir_lowering=False, debug=False)
    x_d = nc.dram_tensor("x", [N, F_IN], F32, kind="ExternalInput")
    w_d = nc.dram_tensor("w", [F_IN, F_OUT], F32, kind="ExternalInput")
    idx_d = nc.dram_tensor("idx", [128, T * 8], I16, kind="ExternalInput")
    dest_d = nc.dram_tensor("dest", [128, T], F32, kind="ExternalInput")
    dsrc_d = nc.dram_tensor("dsrc", [128, T], F32, kind="ExternalInput")
    drep_d = nc.dram_tensor("drep", [128, NWIN * WIN], F32,
                            kind="ExternalInput")
    iota_d = nc.dram_tensor("iota", [128, WIN], F32, kind="ExternalInput")
    if has_bias:
        b_d = nc.dram_tensor("b", [1, F_OUT], F32, kind="ExternalInput")
    out_d = nc.dram_tensor("out", [NWIN * WIN, F_OUT], F32,
                           kind="ExternalOutput")

    max_chunk = max(c[2] for c in chunks)
    assert max_chunk <= 8

    with tile.TileContext(nc) as tc:
        with (
            tc.tile_pool(name="const", bufs=1) as cpool,
            tc.tile_pool(name="gbuf", bufs=8) as gpool,
            tc.tile_pool(name="s", bufs=4) as spool,
            tc.tile_pool(name="ht", bufs=3) as hpool,
            tc.tile_pool(name="osb", bufs=3) as opool,
            tc.tile_pool(name="zpsum", bufs=2, space="PSUM") as zpool,
            tc.tile_pool(name="opsum", bufs=2, space="PSUM") as opsum_pool,
        ):
            nc.gpsimd.load_library(mlp_library)

            # constants
            idx_sb = cpool.tile([128, T * 8], I16)
            nc.sync.dma_start(idx_sb[:], idx_d[:])
            dest_sb = cpool.tile([128, T], F32)
            nc.sync.dma_start(dest_sb[:], dest_d[:])
            dsrc_sb = cpool.tile([128, T], F32)
            nc.sync.dma_start(dsrc_sb[:], dsrc_d[:])
            drep_sb = cpool.tile([128, NWIN * WIN], F32)
            nc.sync.dma_start(drep_sb[:], drep_d[:])
            w_raw = cpool.tile([128, F_OUT], F32)
            nc.sync.dma_start(w_raw[:], w_d[:])
            w_sb = cpool.tile([128, F_OUT], F32R)
            nc.vector.tensor_copy(w_sb[:], w_raw[:])
            iota_sb = cpool.tile([128, WIN], F32)
            nc.sync.dma_start(iota_sb[:], iota_d[:])
            if has_bias:
                b_sb = cpool.tile([1, F_OUT], F32)
                nc.sync.dma_start(b_sb[:], b_d[:])
                ones_sb = cpool.tile([1, WIN], F32)
                nc.vector.memset(ones_sb[:], 1.0)

            x_lo = x_d[:SPLIT, :]
            x_hi = x_d[SPLIT:, :]

            # chunk gather tiles, keyed by chunk id; memset the pool
            # slots once so segment tails never hold uninitialized (NaN)
            # bytes (tails are multiplied by S=0, and 0*NaN=NaN)
            g_tiles = {}

            def gather_chunk(ci):
                l, t0, nt, reg = chunks[ci]
                g = gpool.tile([128, max_chunk, F_IN], F32R, tag="g")
                if reg < nt * TILE:
                    # zero the pad tail the gather won't write: those slots
                    # are multiplied by S=0 and must be finite (0*NaN=NaN)
                    nc.vector.memset(g[:, reg // TILE:nt, :].bitcast(F32), 0)
                nc.gpsimd.dma_gather(
                    g[:, :nt, :],
                    (x_hi if l else x_lo).bitcast(F32R),
                    idx_sb[:, t0 * 8:(t0 + nt) * 8],
                    num_idxs=nt * TILE,
                    num_idxs_reg=reg,
                    elem_size=F_IN,
                )
                g_tiles[ci] = (g, t0, nt)

            # tile t -> (chunk, slot) lookup
            tile_chunk = np.zeros(T, dtype=np.int64)
            for ci, (_, t0, nt, _real) in enumerate(chunks):
                tile_chunk[t0:t0 + nt] = ci

            def do_tile(t, zt, first, last, stage=5):
                ci = tile_chunk[t]
                g, t0, _ = g_tiles[ci]
                s = spool.tile([128, WIN], F32R, tag="s")
                nc.vector.tensor_scalar(
                    s[:], iota_sb[:],
                    dest_sb[:, t:t + 1], dsrc_sb[:, t:t + 1],
                    op0=mybir.AluOpType.is_equal, op1=mybir.AluOpType.mult)
                if stage >= 3:
                    nc.tensor.matmul(
                        zt[:], lhsT=g[:, t - t0, :], rhs=s[:],
                        start=first, stop=last)

            def flush(w, zt):
                # close the accumulation group with a dummy-free stop: the
                # last matmul already ran with stop=False; issue the PSUM
                # read instead. hT = zT * dinv[dst] (free-dim scale).
                ht = hpool.tile([128, WIN], F32R, tag="ht")
                nc.vector.tensor_mul(
                    ht[:], zt[:], drep_sb[:, w * WIN:(w + 1) * WIN])
                # phase C: out[wshard] = relu(hT.T @ W (+ b)), one
                # 128-node tile per half of the 256-wide window
                for half in range(WIN // 128):
                    hs = ht[:, half * 128:(half + 1) * 128]
                    op = opsum_pool.tile([128, F_OUT], F32, tag="op")
                    nc.tensor.matmul(op[:], lhsT=hs, rhs=w_sb[:],
                                     start=True, stop=not has_bias)
                    if has_bias:
                        nc.tensor.matmul(op[:], lhsT=ones_sb[:],
                                         rhs=b_sb[:], start=False,
                                         stop=True)
                    osb = opool.tile([128, F_OUT], F32, tag="osb")
                    nc.scalar.activation(osb[:], op[:],
                                         mybir.ActivationFunctionType.Relu)
                    r0 = w * WIN + half * 128
                    nc.sync.dma_start(out_d[r0:r0 + 128, :], osb[:])

            # main pipeline: per group, gather A+B chunks then process its
            # windows
            n_groups = int(os.environ.get("K_GROUPS", "0")) or len(groups)
            stage = int(os.environ.get("K_STAGE", "5"))

            # group gi covers tiles [tstart[g[0],0] .. end); chunks are in
            # stream order, so just gather chunks in order as needed
            next_chunk = [0]

            def gather_upto(tile_end):
                while (next_chunk[0] < len(chunks)
                       and chunks[next_chunk[0]][1] < tile_end):
                    gather_chunk(next_chunk[0])
                    next_chunk[0] += 1

            for gi, g in enumerate(groups[:n_groups]):
                wlast = g[-1]
                tile_end = int(tstart[wlast, 1] + ntiles[wlast, 1])
                gather_upto(tile_end)
                if stage < 2:
                    continue
                for w in g:
                    zt = zpool.tile([128, WIN], F32, tag="z")
                    wtiles = []
                    for l in (0, 1):
                        t0 = int(tstart[w, l])
                        wtiles.extend(range(t0, t0 + int(ntiles[w, l])))
                    for i, t in enumerate(wtiles):
                        do_tile(t, zt, i == 0, i == len(wtiles) - 1,
                                stage)
                    if stage >= 4:
                        flush(w, zt)

    nc.compile()
    return nc


_CACHE = {}


def kernel(x, edge_index, W, b):
    x = np.ascontiguousarray(np.asarray(x, dtype=np.float32))
    W = np.ascontiguousarray(np.asarray(W, dtype=np.float32))
    b = np.asarray(b, dtype=np.float32)
    edge_index = np.asarray(edge_index)

    per_core, layout = _prep(edge_index)
    has_bias = bool(np.any(b != 0))

    key = (layout["T"], tuple(map(tuple, layout["chunks"])), has_bias,
           os.environ.get("K_GROUPS"), os.environ.get("K_STAGE"))
    if key not in _CACHE:
        _CACHE[key] = _build(layout, has_bias)
    nc = _CACHE[key]

    in_maps = []
    for c in range(NCORES):
        pc = per_core[c]
        m = dict(x=x, w=W, idx=pc["idx"], dest=pc["dest"],
                 dsrc=pc["dsrc"], drep=pc["drep"],
                 iota=np.tile(np.arange(WIN, dtype=np.float32), (128, 1)))
        if has_bias:
            m["b"] = b.reshape(1, F_OUT)
        in_maps.append(m)

    res = run_bass_kernel_spmd(nc, in_maps, core_ids=list(range(NCORES)),
                               trace=bool(int(os.environ.get("K_TRACE", "0"))))
    kernel.last_results = res
    out = np.concatenate(
        [res.results[c]["out"][:SHARD] for c in range(NCORES)], axis=0)
    return out

